# revision 34
# baseline (speedup 1.0000x reference)
"""Trainium2 Bass kernel for the IoU polygon loss (nn_IoUPolyLoss).

Full inputs in, full (scalar) output out. Host shards the 512 polygons
over 8 NeuronCores (64 each: core c -> batch c//2, k-range 64*(c%2))
and performs the index gather (sharding the nb_max_objects axis), so
each core receives only its polygons' vertex features. Rasterization-
free device algorithm: per polygon and scanline, coverage is the
alternating sum of the sorted edge-crossing x-coordinates; the host
combines per-polygon areas into the final loss:
    inter = (area_p + area_g - area_xor) / 2
    union = (area_p + area_g + area_xor) / 2

Device layout per core: partition p = hh*64 + poly (h-half hh), free
dims (side s=2, edge v=16, hl=64); scanline h = hh*64 + hl.

Design notes (all rates HW-measured on TRN2; ~43.7us vs the 50.0us
v1 baseline):
 - host prepares per-core vertex features (the gather by `ind` IS the
   nb_max_objects sharding) as one [128,64] f32 tensor: 128 partition
   rows stripe the input DMA across all 16 SDMA engines (a stride-0
   h-half duplication form splits into only 2 chunks: +1us); py
   constants (pyf f32 / pyh f16) stream in parallel on the Act and
   GpSimd queues
 - both sides processed by ONE 64-wide op chain; trunc toward zero via
   sh = [x<0]-0.5 (one fused tensor_scalar) + RNE MAGIC trick; dall
   computed directly from rolled views (no ver2 copy); dy' fixup fused
   into one scalar_tensor_tensor
 - crossing mask at DVE 2x: y1 broadcast-expanded to (s,v,hl) f16 by
   the Act engine so is_le sees only packed last dims (stride-0 LAST
   dims drop DVE to 1x); crt = roll-XOR at 2x
 - xint chain: t1 = py*A, u1 = t1 + (B+KK) (f32 1x, no fast mode
   exists), Act rounds (+MAGIC) and shifts (-MAGIC-128) emitting f16 X
   in hl-halves pipelined under DVE mask work; mask-apply T0 = X*crt
   at f16 2x (scalar_tensor_tensor is ALWAYS 1x on HW - avoid for big
   tensors)
 - sort-16 (Batcher) + merge-32 (odd-even) in f16 at DVE 2x; ping-pong
   idle lanes copied on Act / DVE-4x-copy in parallel with min/max
 - area sums as odd/even-rank accumulations on the Act engine hidden
   under the merge; final merge round collapses into one STT min with
   accum_out; the host assembles spg/ax from the four raw accumulators
 - ALL scratch lives in 4 consolidated tiles: the tile-pool epilogue
   emits a 5-engine release barrier per tile (~115ns each), so ~35
   tiles cost ~7us of exit drain; 4 tiles cost ~0.5us
 - ONE output DMA in [128,4]-scatter form: 128 small descriptors give
   every SDMA engine real work, so the completion semaphore (16 incs)
   fires ~0.8us after the descriptor; a contiguous single-descriptor
   write leaves 15 engines with empty chunks whose semaphore
   increments wait for a lazy ~4us idle poll (+7-9us tail)
"""
import sys

import numpy as np

try:
    import concourse.bass as bass
except ImportError:
    sys.path.insert(0, "/opt/trn_rl_repo")
    import concourse.bass as bass

import concourse.mybir as mybir
import concourse.tile as tile
import concourse.bacc as bacc
from concourse.bass_utils import run_bass_kernel_spmd

OP = mybir.AluOpType
ACT = mybir.ActivationFunctionType
F32 = mybir.dt.float32
F16 = mybir.dt.float16
F = np.float32

MAGIC = 12582912.0            # 1.5 * 2^23, RN-to-int trick for |x| < 2^22
KK = 0.49545454545454547      # 0.5 - 1/220 margin
M2 = MAGIC + 128.0            # exact in fp32
MAGICM100 = MAGIC - 100.0     # exact in fp32

N_CORES = 8

LAST_RESULTS = None           # BassKernelResults of the most recent run


def _batcher16_pairs():
    n = 16
    rounds = []
    p = 1
    while p < n:
        k = p
        while k >= 1:
            los = []
            j = k % p
            while j <= n - 1 - k:
                for i in range(0, min(k, n - j - k)):
                    if (i + j) // (2 * p) == (i + j + k) // (2 * p):
                        los.append(i + j)
                j += 2 * k
            rounds.append((k, los))
            k //= 2
        p *= 2
    return rounds


def _decompose(idxs):
    n = len(idxs)
    if n == 1:
        return [[1, 1]]
    d = idxs[1] - idxs[0]
    if all(idxs[i] == idxs[0] + i * d for i in range(n)):
        return [[d, n]]
    run = 1
    while run < n and idxs[run] == idxs[0] + run * d:
        run += 1
    assert n % run == 0, f"cannot decompose {idxs}"
    outer = idxs[::run]
    do = outer[1] - outer[0]
    for oi, o in enumerate(outer):
        assert o == outer[0] + oi * do
        for ii in range(run):
            assert idxs[oi * run + ii] == o + ii * d, f"cannot decompose {idxs}"
    return [[do, len(outer)], [d, run]]


def _view(tile_ap, offset, dims):
    return bass.AP(
        tile_ap.tensor,
        tile_ap.offset + offset,
        [list(tile_ap.ap[0])] + [[s, c] for s, c in dims],
    )


def _vdims(idxs, inner=64):
    """AP dims for a set of v-indices (times stride 64, hl inner)."""
    return [[s * 64, c] for s, c in _decompose(idxs)] + [[1, inner]]


# column offsets inside the consolidated tiles
# fA (f32): small per-edge data
FA_RAW, FA_U, FA_VERTS, FA_VER2, FA_DALL, FA_PYF = 0, 64, 128, 192, 256, 320
FA_SH = 384
FA_XK, FA_Z, FA_DS, FA_RT, FA_AT, FA_N1, FA_BT = 448, 480, 512, 544, 576, 608, 640
FA_W = 672
# fB (f32): grid intermediates
FB_T1, FB_U1, FB_WR = 0, 2048, 4096
FB_W = 6144
# hA (f16): everything 16-bit
HA_PYH, HA_Y1E, HA_AGT, HA_CRT, HA_X = 0, 64, 2112, 4160, 6208
HA_T0, HA_T1, HA_TMP, HA_SCR1, HA_M0, HA_M1, HA_SCR2 = (
    8256, 10304, 12352, 13376, 15424, 17472, 19520)
HA_W = 20544
# sm (f32): per-partition scalars
SM_R1O, SM_R1E, SM_R2O, SM_L31, SM_S12, SM_ODD, SM_AR = 0, 1, 2, 3, 4, 5, 6
SM_W = 8


def _build_core_kernel(tc, areas_dram, verts_dram, pyf_dram, pyh_dram):
    nc = tc.nc
    view = _view
    with tc.tile_pool(name="main", bufs=1) as pool:
        fA_t = pool.tile([128, FA_W], F32, tag="fA")
        fB_t = pool.tile([128, FB_W], F32, tag="fB")
        hA_t = pool.tile([128, HA_W], F16, tag="hA")
        sm_t = pool.tile([128, SM_W], F32, tag="sm")
        fA, fB, hA, sm = fA_t[:], fB_t[:], hA_t[:], sm_t[:]

        # ---------------- input DMAs, three parallel queues.  verts is
        # host-duplicated to [128,64]: the 128 partition rows stripe the
        # transfer across all 16 SDMA engines (a stride-0 dup form costs
        # +1us: it splits into only 2 chunks -> 2 engines)
        nc.sync.dma_start(out=view(fA, FA_RAW, [(1, 64)]), in_=verts_dram)
        nc.scalar.dma_start(out=view(fA, FA_PYF, [(1, 64)]), in_=pyf_dram)
        nc.gpsimd.dma_start(out=view(hA, HA_PYH, [(1, 64)]), in_=pyh_dram)

        # ---------------- trunc(x)+100 both sides at once:
        # sh = [x<0] - 0.5, u = x + sh, verts = RNE(u) + 100 (MAGIC)
        rawv = view(fA, FA_RAW, [(1, 64)])
        shv = view(fA, FA_SH, [(1, 64)])
        nc.vector.tensor_scalar(shv, rawv, 0.0, 0.5, OP.is_lt, OP.subtract)
        uv = view(fA, FA_U, [(1, 64)])
        nc.vector.tensor_tensor(uv, rawv, shv, OP.add)
        vertsv = view(fA, FA_VERTS, [(1, 64)])
        nc.vector.tensor_scalar(vertsv, uv, MAGIC, MAGICM100,
                                OP.add, OP.subtract)
        # xk = x1 + 100 + KK (the +KK margin folded into B)
        PV = [(16, 2), (1, 16)]
        nc.vector.tensor_scalar(view(fA, FA_XK, PV),
                                view(fA, FA_VERTS, [(32, 2), (2, 16)]),
                                KK, None, OP.add)

        # ---------------- edges: dall = next - cur directly (main + wrap)
        nc.vector.tensor_tensor(view(fA, FA_DALL, [(32, 2), (2, 15), (1, 2)]),
                                view(fA, FA_VERTS + 2, [(32, 2), (2, 15), (1, 2)]),
                                view(fA, FA_VERTS, [(32, 2), (2, 15), (1, 2)]),
                                OP.subtract)
        nc.vector.tensor_tensor(view(fA, FA_DALL + 30, [(32, 2), (1, 2)]),
                                view(fA, FA_VERTS, [(32, 2), (1, 2)]),
                                view(fA, FA_VERTS + 30, [(32, 2), (1, 2)]),
                                OP.subtract)

        # y1 broadcast-expand to (s,v,hl) f16 on the Act engine (feeds
        # the 2x is_le crossing mask); runs while DVE does edge math
        nc.scalar.activation(
            view(hA, HA_Y1E, [(1024, 2), (64, 16), (1, 64)]),
            view(fA, FA_VERTS + 1, [(32, 2), (2, 16), (0, 64)]), ACT.Copy)

        # A = dx/dy', Bkk = xk - y1*A   (per-edge, 32-wide both sides)
        dyv = view(fA, FA_DALL + 1, [(32, 2), (2, 16)])
        dxv = view(fA, FA_DALL, [(32, 2), (2, 16)])
        y1v = view(fA, FA_VERTS + 1, [(32, 2), (2, 16)])
        nc.vector.scalar_tensor_tensor(view(fA, FA_DS, PV), dyv, 0.0,
                                       dyv, OP.is_equal, OP.add)
        nc.vector.reciprocal(view(fA, FA_RT, [(1, 32)]),
                             view(fA, FA_DS, [(1, 32)]))
        nc.vector.tensor_tensor(view(fA, FA_AT, PV), view(fA, FA_RT, PV),
                                dxv, OP.mult)
        nc.vector.tensor_tensor(view(fA, FA_N1, PV), y1v,
                                view(fA, FA_AT, PV), OP.mult)
        nc.vector.scalar_tensor_tensor(view(fA, FA_BT, PV),
                                       view(fA, FA_N1, PV), -1.0,
                                       view(fA, FA_XK, PV),
                                       OP.mult, OP.add)

        # ---------------- grid stage, free = (s2, v16, hl64) = 2048
        # xint+KK = py*A + Bkk in f32 (two hl-halves to pipeline the Act
        # rounding); Act: wr = u1 + MAGIC, X = wr - (MAGIC+128) -> f16
        GH = [(64, 32), (1, 32)]             # one hl-half, flat (s*v, hl)
        for h in (0, 32):
            nc.vector.tensor_tensor(
                view(fB, FB_T1 + h, [(1024, 2), (64, 16), (1, 32)]),
                view(fA, FA_PYF + h, [(0, 2), (0, 16), (1, 32)]),
                view(fA, FA_AT, [(16, 2), (1, 16), (0, 32)]), OP.mult)
            nc.vector.tensor_tensor(view(fB, FB_U1 + h, GH),
                                    view(fB, FB_T1 + h, GH),
                                    view(fA, FA_BT, [(1, 32), (0, 32)]),
                                    OP.add)
            nc.scalar.activation(view(fB, FB_WR + h, GH),
                                 view(fB, FB_U1 + h, GH),
                                 ACT.Copy, bias=MAGIC)
            nc.scalar.activation(view(hA, HA_X + h, GH),
                                 view(fB, FB_WR + h, GH),
                                 ACT.Copy, bias=-M2)

        # ---------------- crossing mask at 2x: agt = (y1 <= py),
        # crt = agt XOR roll(agt); then T0 = X * crt (f16 2x)
        GD3 = [(1024, 2), (64, 16), (1, 64)]
        nc.vector.tensor_tensor(view(hA, HA_AGT, GD3),
                                view(hA, HA_Y1E, GD3),
                                view(hA, HA_PYH, [(0, 2), (0, 16), (1, 64)]),
                                OP.is_le)
        nc.vector.tensor_tensor(
            view(hA, HA_CRT, [(1024, 2), (64, 15), (1, 64)]),
            view(hA, HA_AGT + 64, [(1024, 2), (64, 15), (1, 64)]),
            view(hA, HA_AGT, [(1024, 2), (64, 15), (1, 64)]), OP.not_equal)
        nc.vector.tensor_tensor(
            view(hA, HA_CRT + 15 * 64, [(1024, 2), (1, 64)]),
            view(hA, HA_AGT, [(1024, 2), (1, 64)]),
            view(hA, HA_AGT + 15 * 64, [(1024, 2), (1, 64)]), OP.not_equal)

        GH3 = [(1024, 2), (64, 16), (1, 32)]
        for h in (0, 32):
            nc.vector.tensor_tensor(view(hA, HA_T0 + h, GH3),
                                    view(hA, HA_X + h, GH3),
                                    view(hA, HA_CRT + h, GH3), OP.mult)

        # ---------------- sort-16 along v (ascending), fp16
        bufs = [HA_T0, HA_T1]
        cur = 0
        for k, los in _batcher16_pairs():
            npairs = len(los)
            touched = sorted(los + [l + k for l in los])
            idle = [i for i in range(16) if i not in touched]
            C = bufs[cur]
            lo_dims = [(1024, 2)] + _vdims(los)
            lo_src = view(hA, C + los[0] * 64, lo_dims)
            hi_src = view(hA, C + (los[0] + k) * 64, lo_dims)
            N = bufs[1 - cur]
            nc.vector.tensor_tensor(view(hA, N + los[0] * 64, lo_dims),
                                    lo_src, hi_src, OP.min)
            nc.vector.tensor_tensor(view(hA, N + (los[0] + k) * 64, lo_dims),
                                    lo_src, hi_src, OP.max)
            if len(idle) >= 2 * npairs:
                # big idle sets: half on the DVE (4x fp16 copy), half on
                # the Act engine, in parallel with the round's min/max
                splits = {
                    (0, 3, 4, 7, 8, 11, 12, 15): ([0, 4, 8, 12],
                                                  [3, 7, 11, 15]),
                    (0, 1, 6, 7, 8, 9, 14, 15): ([0, 1, 14, 15],
                                                 [6, 7, 8, 9]),
                    (0, 1, 2, 3, 12, 13, 14, 15): ([0, 1, 2, 3],
                                                   [12, 13, 14, 15]),
                }
                dve_idle, act_idle = splits[tuple(idle)]
                ddims = [(1024, 2)] + _vdims(dve_idle)
                nc.vector.tensor_copy(view(hA, N + dve_idle[0] * 64, ddims),
                                      view(hA, C + dve_idle[0] * 64, ddims))
                adims = [(1024, 2)] + _vdims(act_idle)
                nc.scalar.activation(view(hA, N + act_idle[0] * 64, adims),
                                     view(hA, C + act_idle[0] * 64, adims),
                                     ACT.Copy)
            elif idle:
                idims = [(1024, 2)] + _vdims(idle)
                nc.scalar.activation(view(hA, N + idle[0] * 64, idims),
                                     view(hA, C + idle[0] * 64, idims),
                                     ACT.Copy)
            cur = 1 - cur
        S = bufs[cur]                        # sorted, fp16, (s, v16, hl)

        # ---------------- area_p + area_g on Act: odd-rank sum minus
        # even-rank sum (overlaps with the merge running on the DVE)
        ar1o = view(sm, SM_R1O, [(1, 1)])
        ar1e = view(sm, SM_R1E, [(1, 1)])
        ar2o = view(sm, SM_R2O, [(1, 1)])
        ODD = [(128, 16), (1, 64)]                    # every 2nd sv-lane
        nc.scalar.activation(view(hA, HA_SCR1 + 64, ODD),
                             view(hA, S + 64, ODD), ACT.Copy,
                             accum_out=ar1o)
        nc.scalar.activation(view(hA, HA_SCR1, ODD),
                             view(hA, S, ODD), ACT.Copy,
                             accum_out=ar1e)

        # ---------------- merge-32: Batcher odd-even merge.  S is
        # [pred asc ++ gt asc] in flat sv-lane order, so round 0 reads S
        # directly.  rounds: (distance, lo-indices, idle); None = in-place
        OEM = [
            (16, list(range(16)), []),
            (8, list(range(8, 16)), None),            # in-place round
            (4, [4, 5, 6, 7, 12, 13, 14, 15, 20, 21, 22, 23],
             [0, 1, 2, 3, 28, 29, 30, 31]),
            (2, [2, 3, 6, 7, 10, 11, 14, 15, 18, 19, 22, 23, 26, 27],
             [0, 1, 30, 31]),
        ]
        mbufs = [HA_M0, HA_M1]
        C, mcur = S, 0
        for d, los, idle in OEM:
            ldims = _vdims(los)
            lo_src = view(hA, C + los[0] * 64, ldims)
            hi_src = view(hA, C + (los[0] + d) * 64, ldims)
            if idle is None:                 # in-place (C is a merge buf)
                tmp_ap = view(hA, HA_TMP, [[64, len(los)], [1, 64]])
                nc.vector.tensor_tensor(tmp_ap, lo_src, hi_src, OP.max)
                nc.vector.tensor_tensor(lo_src, lo_src, hi_src, OP.min)
                nc.vector.tensor_copy(hi_src, tmp_ap)
            else:
                N = mbufs[mcur]
                nc.vector.tensor_tensor(view(hA, N + los[0] * 64, ldims),
                                        lo_src, hi_src, OP.min)
                nc.vector.tensor_tensor(view(hA, N + (los[0] + d) * 64, ldims),
                                        lo_src, hi_src, OP.max)
                if idle:
                    idims = _vdims(idle)
                    nc.scalar.activation(view(hA, N + idle[0] * 64, idims),
                                         view(hA, C + idle[0] * 64, idims),
                                         ACT.Copy)
                C = N
                mcur = 1 - mcur

        # ---------------- fused final round + output:
        # the last OEM round (d=1) collapses into ONE scalar_tensor_tensor
        # MIN with accum_out; the host assembles
        #   ax = 2*(ar2o + l31) - (ar1o + ar1e),  spg = ar1o - ar1e
        # from the four raw per-partition accumulators.  ONE output DMA in
        # the [128,4]-scatter form: its 128 small descriptors spread real
        # work over all 16 SDMA engines, so the completion semaphore fires
        # ~2.8us after the descriptor (a contiguous single-descriptor
        # write leaves 15 engines with empty chunks whose semaphore
        # increments wait for a lazy ~4us idle poll: +7-9us tail).
        l31 = view(sm, SM_L31, [(1, 1)])
        nc.vector.tensor_reduce(l31, view(hA, C + 31 * 64, [(1, 64)]),
                                axis=mybir.AxisListType.X, op=OP.add)
        modd = [(128, 15), (1, 64)]
        nc.vector.scalar_tensor_tensor(
            view(hA, HA_SCR2, [(64, 15), (1, 64)]),
            view(hA, C + 64, modd), 0.0, view(hA, C + 2 * 64, modd),
            OP.add, OP.min, accum_out=ar2o)
        nc.sync.dma_start(out=bass.AP(areas_dram.tensor, areas_dram.offset,
                                      [[4, 128], [1, 4]]),
                          in_=view(sm, SM_R1O, [(1, 4)]))


_CACHED_NC = None


def _get_nc():
    global _CACHED_NC
    if _CACHED_NC is not None:
        return _CACHED_NC
    nc = bacc.Bacc("TRN2", target_bir_lowering=False, debug=False,
                   num_devices=N_CORES)
    verts = nc.dram_tensor("verts", [128, 64], F32, kind="ExternalInput")
    pyf = nc.dram_tensor("pyf", [128, 64], F32, kind="ExternalInput")
    pyh = nc.dram_tensor("pyh", [128, 64], F16, kind="ExternalInput")
    areas = nc.dram_tensor("areas", [128, 4], F32, kind="ExternalOutput")
    with tile.TileContext(nc) as tc:
        _build_core_kernel(tc, areas.ap(), verts.ap(), pyf.ap(), pyh.ap())
    nc.compile()
    _CACHED_NC = nc
    return nc


def kernel(output, mask, ind, target):
    global LAST_RESULTS
    output = np.asarray(output)
    mask = np.asarray(mask)
    ind = np.asarray(ind)
    target = np.asarray(target)
    B, C, H, W = output.shape

    # py constants, identical on every core: py = hh*64 + hl
    hh = (np.arange(128) // 64 * 64).astype(F)[:, None]
    hl = np.arange(64, dtype=F)[None, :]
    PYF = np.ascontiguousarray(hh + hl, dtype=np.float32)
    PYH = PYF.astype(np.float16)

    # ---- host-side sharding: each core gets its 64 polygons' pred
    # features (gathered by ind) and gt features, pred cols 0..31 /
    # gt cols 32..63; the device DMA duplicates rows for the h-halves
    in_maps = []
    for c in range(N_CORES):
        b, k0 = c // 2, 64 * (c % 2)
        idx = np.asarray(ind[b, k0:k0 + 64]).astype(np.int64)
        pv = output[b].reshape(C, H * W)[:, idx].T          # [64, 32]
        gv = target[b][:, k0:k0 + 64].T                     # [64, 32]
        raw64 = np.concatenate([pv, gv], axis=1).astype(F)  # [64, 64]
        raw = np.ascontiguousarray(np.tile(raw64, (2, 1)))  # [128, 64]
        in_maps.append({"verts": raw, "pyf": PYF, "pyh": PYH})

    nc = _get_nc()
    res = run_bass_kernel_spmd(nc, in_maps, core_ids=list(range(N_CORES)))
    LAST_RESULTS = res

    # ---- host-side gather + final scalar assembly
    spg = np.zeros((B, 128), np.float32)     # area_p + area_g per poly
    ax = np.zeros((B, 128), np.float32)      # area_xor per poly
    for c in range(N_CORES):
        b, k0 = c // 2, 64 * (c % 2)
        a = res.results[c]["areas"].astype(np.float64)       # [128, 4]
        spg_p = a[:, 0] - a[:, 1]                 # odd - even rank sums
        ax_p = 2.0 * (a[:, 2] + a[:, 3]) - (a[:, 0] + a[:, 1])
        spg[b, k0:k0 + 64] = (spg_p[:64] + spg_p[64:]).astype(F)
        ax[b, k0:k0 + 64] = (ax_p[:64] + ax_p[64:]).astype(F)
    inter = ((spg - ax) / 2).astype(F)
    union = ((spg + ax) / 2).astype(F)
    iou = (inter / (union + F(1e-4))).astype(F)
    mm = mask.astype(F)
    loss = F(F(1.0) - (iou * mm).sum(dtype=F) / (mm.sum(dtype=F) + F(1e-4)))
    return np.asarray(loss, dtype=np.float32)
